# revision 1
# baseline (speedup 1.0000x reference)
"""ClashLoss kernel for Trainium2 (8 NeuronCores, batch-parallel).

Math: for each batch b, count pairs (n, m), n != m, with
    dist(n, m) < radii[n] + radii[m]   and   dist(n, m) > EPS.
Using s_n = |c_n|^2 - r_n^2, the clash condition dist^2 < (r_n + r_m)^2 is
    G[n, m] = dot(c_n, c_m) + r_n r_m - s_n/2 - s_m/2 > 0,
i.e. a 6-dim dot product u_n . v_m with
    u_n = (x, y, z, r_n, -s_n/2, 1)     (matmul stationary side)
    v_m = (x, y, z, r_m, 1, -s_m/2)     (matmul moving side)
The PE computes G tile-by-tile into PSUM; ACT (Sign + accumulate) and DVE
(tensor_scalar is_gt + accumulate) count positives per 512x512 super-block.
Symmetry: only upper-triangular super-blocks are computed (weight 2), the
diagonal super-blocks once (weight 1).  The diagonal n == m has
G[n,n] = 2 r_n^2 > 0 always, so exactly N diagonal hits are subtracted on
the host.

Raw-bass implementation (not Tile): fp32 matmuls only support a single
sync-wait in codegen, so semaphores are placed by hand -- at most one wait
per matmul, standalone wait instructions elsewhere.
"""

import numpy as np

N = 4096
B = 8
K = 6  # augmented dot-product length
SUPER = 512  # super-block edge (4 PSUM banks wide)
NSUP = N // SUPER  # 8
ROWT = 128  # rows per matmul (output partitions)
CHUNK_FD = SUPER * 2  # 1024 f32 = 2 PSUM banks; 4 chunks rotate
NCHUNKS = 4
MM_PER_UNIT = 2  # each unit = one chunk = 2 row-tile matmuls
UNITS_PER_SUPER = 2
EPS = 1e-8

# super-block schedule: (R, C, weight); R <= C
SCHEDULE = [(r, c, 1 if r == c else 2) for r in range(NSUP) for c in range(r, NSUP)]
NSLOTS = len(SCHEDULE)  # 36

# engine assignment: interleave ACT/DVE (Bresenham spread) so the two
# consumer engines run concurrently on alternating PSUM chunk buffers;
# ACT is a bit faster per chunk so it takes 20 of 36 supers.
NUNITS = NSLOTS * UNITS_PER_SUPER  # 72
N_ACT_TARGET = 40
ENGINE = [
    "act"
    if (i + 1) * N_ACT_TARGET // NUNITS > i * N_ACT_TARGET // NUNITS
    else "dve"
    for i in range(NUNITS)
]
# compact per-engine slot index for each unit
SLOT_IDX = []
_na = _nd = 0
for _i in range(NUNITS):
    if ENGINE[_i] == "act":
        SLOT_IDX.append(_na)
        _na += 1
    else:
        SLOT_IDX.append(_nd)
        _nd += 1
N_ACT, N_DVE = _na, _nd

_CACHE = {}


def _build(repeat=1, mm_dtype="float32r"):
    """Build the raw-bass SPMD program (same program for all cores).

    repeat > 1 re-runs the whole super-block schedule (for differential
    wall-clock timing); the counts are simply overwritten each pass.
    """
    import concourse.bass as bass
    from concourse import mybir

    nc = bass.Bass("TRN2", target_bir_lowering=False, debug=False)
    f32 = mybir.dt.float32
    mdt = getattr(mybir.dt, mm_dtype)

    u_dram = nc.dram_tensor("u6", [K, N], mdt, kind="ExternalInput").ap()
    v_dram = nc.dram_tensor("v6", [K, N], mdt, kind="ExternalInput").ap()
    out_dram = nc.dram_tensor(
        "counts", [128, N_ACT + N_DVE], f32, kind="ExternalOutput"
    ).ap()

    # flattened unit schedule over repeats: unit -> (R, C, half)
    usched = []
    for R, C, _w in SCHEDULE:
        usched.append((R, C, 0))
        usched.append((R, C, 1))
    gsched = usched * repeat
    gengine = ENGINE * repeat
    gslot = SLOT_IDX * repeat
    ntot = len(gsched)

    # consumer bookkeeping: for global super g, which engine consumes it and
    # the cumulative per-engine consumption count up to and including g.
    cons_count = []  # (engine, count_after_g)
    na = nd = 0
    for g in range(ntot):
        if gengine[g] == "act":
            na += 1
            cons_count.append(("act", na))
        else:
            nd += 1
            cons_count.append(("dve", nd))
    n_act_total, n_dve_total = na, nd

    with (
        nc.sbuf_tensor([K, N], mdt) as u_sb,
        nc.sbuf_tensor([K, N], mdt) as v_sb,
        nc.sbuf_tensor([128, max(1, N_ACT)], f32) as act_slots,
        nc.sbuf_tensor([128, max(1, N_DVE)], f32) as dve_slots,
        nc.sbuf_tensor([128, NUNITS + 3], f32) as act_dummy,
        nc.sbuf_tensor([128, NUNITS], f32) as dve_dummy,
        nc.psum_tensor([128, CHUNK_FD], f32) as chunk0,
        nc.psum_tensor([128, CHUNK_FD], f32) as chunk1,
        nc.psum_tensor([128, CHUNK_FD], f32) as chunk2,
        nc.psum_tensor([128, CHUNK_FD], f32) as chunk3,
        nc.semaphore("DMA_IN") as s_in,
        nc.semaphore("PROD") as s_prod,
        nc.semaphore("CACT") as s_cact,
        nc.semaphore("CDVE") as s_cdve,
        nc.semaphore("DMA_OUT") as s_out,
        nc.Block() as block,
    ):
        chunks = [chunk0, chunk1, chunk2, chunk3]

        @block.sync
        def _(sync):
            sync.dma_start(out=u_sb[:, :], in_=u_dram).then_inc(s_in, 16)
            sync.dma_start(out=v_sb[:, :], in_=v_dram).then_inc(s_in, 16)
            sync.wait_ge(s_cact, n_act_total)
            sync.wait_ge(s_cdve, n_dve_total)
            sync.dma_start(
                out=out_dram[:, 0:N_ACT], in_=act_slots[:, :]
            ).then_inc(s_out, 16)
            sync.dma_start(
                out=out_dram[:, N_ACT : N_ACT + N_DVE], in_=dve_slots[:, :]
            ).then_inc(s_out, 16)
            sync.wait_ge(s_out, 32)

        @block.tensor
        def _(tensor):
            for g in range(ntot):
                R, C, half = gsched[g]
                chunk = chunks[g % NCHUNKS]
                if g == 0:
                    tensor.wait_ge(s_in, 32)
                if g >= NCHUNKS:
                    eng, cnt = cons_count[g - NCHUNKS]
                    tensor.wait_ge(s_cact if eng == "act" else s_cdve, cnt)
                for j in range(MM_PER_UNIT):
                    jt = half * MM_PER_UNIT + j  # row-tile within super
                    mm = nc.tensor.matmul(
                        chunk[:, j * SUPER : (j + 1) * SUPER],
                        lhsT=u_sb[
                            :, R * SUPER + jt * ROWT : R * SUPER + (jt + 1) * ROWT
                        ],
                        rhs=v_sb[:, C * SUPER : (C + 1) * SUPER],
                        start=True,
                        stop=True,
                    )
                    if j == MM_PER_UNIT - 1:
                        mm.then_inc(s_prod, 1)

        @block.scalar
        def _(scalar):
            # warm the Sign activation table while the input DMA is in
            # flight (table load is ~2.7us and would otherwise serialize
            # in front of the first real chunk).
            nc.scalar.memzero(act_dummy.ap()[:, NUNITS : NUNITS + 1])
            nc.scalar.activation(
                out=act_dummy.ap()[:, NUNITS + 1 : NUNITS + 2],
                in_=act_dummy.ap()[:, NUNITS : NUNITS + 1],
                func=mybir.ActivationFunctionType.Sign,
                accum_out=act_dummy.ap()[:, NUNITS + 2 : NUNITS + 3],
            )
            for g in range(ntot):
                if gengine[g] != "act":
                    continue
                i = g % NUNITS
                chunk = chunks[g % NCHUNKS]
                scalar.wait_ge(s_prod, g + 1)
                nc.scalar.activation(
                    out=act_dummy.ap()[:, i : i + 1].broadcast_to((128, CHUNK_FD)),
                    in_=chunk[:, :],
                    func=mybir.ActivationFunctionType.Sign,
                    accum_out=act_slots[:, gslot[g] : gslot[g] + 1],
                ).then_inc(s_cact, 1)

        @block.vector
        def _(vector):
            for g in range(ntot):
                if gengine[g] != "dve":
                    continue
                i = g % NUNITS
                chunk = chunks[g % NCHUNKS]
                vector.wait_ge(s_prod, g + 1)
                nc.vector.tensor_scalar(
                    out=dve_dummy.ap()[:, i : i + 1].broadcast_to((128, CHUNK_FD)),
                    in0=chunk[:, :],
                    scalar1=0.0,
                    scalar2=None,
                    op0=mybir.AluOpType.is_gt,
                    op1=mybir.AluOpType.add,
                    accum_out=dve_slots[:, gslot[g] : gslot[g] + 1],
                ).then_inc(s_cdve, 1)

    return nc


def _prep_inputs(coords, atom_types, vdw_radii):
    """Host-side shard prep: per-batch u6/v6 [6, N] f32 arrays."""
    coords = np.asarray(coords, dtype=np.float32)  # [B, N, 3]
    atom_types = np.asarray(atom_types).astype(np.int64)  # [B, N]
    vdw_radii = np.asarray(vdw_radii, dtype=np.float32)  # [T]
    r = vdw_radii[atom_types]  # [B, N] f32 gather
    sq = np.einsum("bnd,bnd->bn", coords, coords, dtype=np.float32).astype(np.float32)
    s = (sq - r * r).astype(np.float32)
    in_maps = []
    for b in range(B):
        u = np.empty((K, N), np.float32)
        v = np.empty((K, N), np.float32)
        u[0:3] = coords[b].T
        v[0:3] = coords[b].T
        u[3] = r[b]
        v[3] = r[b]
        u[4] = -0.5 * s[b]
        v[4] = 1.0
        u[5] = 1.0
        v[5] = -0.5 * s[b]
        in_maps.append({"u6": u, "v6": v})
    return in_maps


def _combine(results):
    """Host-side gather: per-core count slots -> scalar loss."""
    chunk_elems = 128 * CHUNK_FD
    total = 0.0
    for b in range(B):
        counts = np.asarray(results[b]["counts"], np.float64)
        act = counts[:, :N_ACT].sum(axis=0)
        dve = counts[:, N_ACT:].sum(axis=0)
        cnt_b = 0.0
        for i in range(NUNITS):
            w = SCHEDULE[i // UNITS_PER_SUPER][2]
            if ENGINE[i] == "act":
                cnt = (chunk_elems + act[SLOT_IDX[i]]) / 2.0  # positives from sign-sum
            else:
                cnt = dve[SLOT_IDX[i]]
            cnt_b += w * cnt
        cnt_b -= N  # remove diagonal (G[n,n] = 2 r^2 > 0 always)
        total += (cnt_b / 2.0) / N
    return np.float32(total / B)


def kernel(coords, atom_types, vdw_radii):
    import sys

    if "/opt/trn_rl_repo" not in sys.path:
        sys.path.insert(0, "/opt/trn_rl_repo")
    from concourse.bass_utils import run_bass_kernel_spmd

    if "nc" not in _CACHE:
        _CACHE["nc"] = _build()
    nc = _CACHE["nc"]

    in_maps = _prep_inputs(coords, atom_types, vdw_radii)
    res = run_bass_kernel_spmd(nc, in_maps, core_ids=list(range(B)))
    return _combine(res.results)


if __name__ == "__main__":
    import sys

    sys.path.insert(0, "/root/problem")
    import reference as ref

    inputs = ref.setup_inputs()
    out = kernel(**{k: np.asarray(v) for k, v in inputs.items()})
    print("kernel output:", out)



# revision 2
# speedup vs baseline: 5.7649x; 5.7649x over previous
"""ClashLoss kernel for Trainium2 (8 NeuronCores, batch-parallel).

Math: for each batch b, count pairs (n, m) with
    dist(n, m) < r_n + r_m   and   dist(n, m) > EPS.
Equivalent test (squared form, h = (|c|^2 - r^2)/2):
    x_n x_m + y_n y_m + z_n z_m + r_n r_m - h_n  >  h_m .

The execution backend charges a large, nearly size-independent cost per
engine INSTRUCTION, so the kernel minimizes instruction count rather
than classical FLOP/byte rooflines.  The whole 4096x4096 pair matrix is
evaluated on the DVE engine in 4 "giant chains".  Each chain covers
T=8 row-blocks (1024 rows x 4096 cols) in 9 instructions operating on
[128, 8, 4096] access patterns (f16 work buffers, 64 KB/partition):

    A  = Ux (x) Vx          broadcast outer-product  (tensor_tensor mult)
    P  = Uy (x) Vy ; A += P
    P  = Uz (x) Vz ; A += P
    P  = Ur (x) Vr ; A += P
    A -= H                  (h_n, broadcast along columns)
    cnt[p] = sum_j 1[A > W] (scalar_tensor_tensor is_gt + accum_out)

U-side operands are per-(partition, row-block) scalars broadcast along
columns via stride-0 AP dims; V-side operands are row-replicated vectors
broadcast across row-blocks.  f16 work buffers flip only pairs whose
clash margin is below ~1e-3 of scale; measured end-to-end rel err vs the
f32 reference is ~1.3e-3 (gate: 2e-2).

The diagonal n == m always tests positive (margin 2 r_n^2 > 0), so N is
subtracted on the host; the final scalar is the batch-mean of
(count - N) / 2 / N computed host-side from the 8 per-core [128, 4]
count slots.
"""

import numpy as np

N = 4096
B = 8
T = 8            # row-blocks per chain
NCHAIN = 32 // T  # 4
_CACHE = {}


def _build(repeat=1):
    import concourse.bass as bass
    from concourse import mybir

    nc = bass.Bass("TRN2", target_bir_lowering=False, debug=False)
    f16 = mybir.dt.float16
    f32 = mybir.dt.float32

    vpack_d = nc.dram_tensor("vpack", [128, 5 * N], f16, kind="ExternalInput").ap()
    upack_d = nc.dram_tensor("upack", [128, 5 * 32], f16, kind="ExternalInput").ap()
    cnt_d = nc.dram_tensor("cnt", [128, NCHAIN], f32, kind="ExternalOutput").ap()

    with (
        nc.sbuf_tensor([128, 5 * N], f16) as vpack,
        nc.sbuf_tensor([128, 5 * 32], f16) as upack,
        nc.sbuf_tensor([128, T * N], f16) as A,
        nc.sbuf_tensor([128, T * N], f16) as P,
        nc.sbuf_tensor([128, NCHAIN], f32) as cnt,
        nc.semaphore("SIN") as s_in,
        nc.semaphore("SDONE") as s_done,
        nc.semaphore("SOUT") as s_out,
        nc.Block() as block,
    ):
        # vpack columns: [x | y | z | r | h], each N wide, row-replicated.
        # upack columns: [x | y | z | r | h], each 32 wide; upack[p, k*32+t]
        # is component k of atom 128*t + p.
        def vap(k):
            return vpack.ap()[:, k * N:(k + 1) * N].unsqueeze(1).broadcast_to((128, T, N))

        def uap(k, c):
            lo = k * 32 + T * c
            return upack.ap()[:, lo:lo + T].unsqueeze(2).broadcast_to((128, T, N))

        @block.sync
        def _(sync):
            sync.dma_start(out=vpack[:, :], in_=vpack_d).then_inc(s_in, 16)
            sync.dma_start(out=upack[:, :], in_=upack_d).then_inc(s_in, 16)
            sync.wait_ge(s_done, repeat)
            sync.dma_start(out=cnt_d, in_=cnt[:, :]).then_inc(s_out, 16)
            sync.wait_ge(s_out, 16)

        @block.vector
        def _(vector):
            vector.wait_ge(s_in, 32)
            mult, add, sub = (
                mybir.AluOpType.mult,
                mybir.AluOpType.add,
                mybir.AluOpType.subtract,
            )
            for rep in range(repeat):
                for c in range(NCHAIN):
                    o = A.ap()[:, :].rearrange("p (t j) -> p t j", t=T)
                    po = P.ap()[:, :].rearrange("p (t j) -> p t j", t=T)
                    nc.vector.tensor_tensor(out=o, in0=uap(0, c), in1=vap(0), op=mult)
                    nc.vector.tensor_tensor(out=po, in0=uap(1, c), in1=vap(1), op=mult)
                    nc.vector.tensor_tensor(out=o, in0=o, in1=po, op=add)
                    nc.vector.tensor_tensor(out=po, in0=uap(2, c), in1=vap(2), op=mult)
                    nc.vector.tensor_tensor(out=o, in0=o, in1=po, op=add)
                    nc.vector.tensor_tensor(out=po, in0=uap(3, c), in1=vap(3), op=mult)
                    nc.vector.tensor_tensor(out=o, in0=o, in1=po, op=add)
                    nc.vector.tensor_tensor(out=o, in0=o, in1=uap(4, c), op=sub)
                    ins = nc.vector.scalar_tensor_tensor(
                        out=o, in0=o, scalar=0.0, in1=vap(4),
                        op0=add, op1=mybir.AluOpType.is_gt,
                        accum_out=cnt[:, c:c + 1])
                    if c == NCHAIN - 1:
                        ins.then_inc(s_done, 1)
    return nc


def _prep_inputs(coords, atom_types, vdw_radii):
    """Host-side shard prep: per-batch f16 vpack/upack arrays."""
    coords = np.asarray(coords, dtype=np.float32)   # [B, N, 3]
    atom_types = np.asarray(atom_types).astype(np.int64)
    vdw = np.asarray(vdw_radii, dtype=np.float32)
    r = vdw[atom_types]                              # [B, N]
    sq = np.einsum("bnd,bnd->bn", coords, coords).astype(np.float32)
    h = ((sq - r * r) / 2.0).astype(np.float32)
    in_maps = []
    for b in range(coords.shape[0]):
        x, y, z = coords[b, :, 0], coords[b, :, 1], coords[b, :, 2]
        comps = (x, y, z, r[b], h[b])
        vp = np.empty((128, 5 * N), np.float16)
        up = np.empty((128, 5 * 32), np.float16)
        for k, arr in enumerate(comps):
            a16 = arr.astype(np.float16)
            vp[:, k * N:(k + 1) * N] = a16[None, :]
            up[:, k * 32:(k + 1) * 32] = a16.reshape(32, 128).T
        in_maps.append({"vpack": vp, "upack": up})
    return in_maps


def _combine(results):
    """Host-side gather: per-core count slots -> scalar loss."""
    total = 0.0
    for b in range(len(results)):
        c = np.asarray(results[b]["cnt"], np.float64).sum()
        total += (c - N) / 2.0 / N
    return np.float32(total / len(results))


def kernel(coords, atom_types, vdw_radii):
    import sys

    if "/opt/trn_rl_repo" not in sys.path:
        sys.path.insert(0, "/opt/trn_rl_repo")
    from concourse.bass_utils import run_bass_kernel_spmd

    if "nc" not in _CACHE:
        _CACHE["nc"] = _build()
    nc = _CACHE["nc"]

    in_maps = _prep_inputs(coords, atom_types, vdw_radii)
    res = run_bass_kernel_spmd(nc, in_maps, core_ids=list(range(B)))
    return _combine(res.results)


if __name__ == "__main__":
    import sys

    sys.path.insert(0, "/root/problem")
    import reference as ref

    inputs = ref.setup_inputs()
    out = kernel(**{k: np.asarray(v) for k, v in inputs.items()})
    print("kernel output:", out)
